# revision 8
# baseline (speedup 1.0000x reference)
"""Multi-head attention forward on 8 Trainium2 NeuronCores.

Problem (all shapes hardcoded): B=2, S=2048, D=1024, H=16, HD=64
    q = relu(x @ Wq + bq); k = relu(x @ Wk + bk); v = relu(x @ Wv + bv)
    attn = softmax(q k^T / sqrt(HD)) per (batch, head)
    out = relu((attn @ v) @ Wo + bo)

Sharding: head-parallel for QKV+attention (2 heads per core, both batches),
then AllToAlls re-shard the per-head context to a per-token shard and each
core runs the full output projection for its 512 tokens. Host reassembles.

Device schedule (per core):
  - Q^T, K^T ([64, 4096] per head) and V_aug ([128 tokens, 64 V cols + 64
    ones cols] per head block) via bf16 matmuls against x^T, fp32 PSUM.
  - scores computed transposed S^T[k, q] = K^T.T @ Q^T per head (K_c=64);
    exp on ACT straight from PSUM with the 1/8 scale folded in (scores are
    O(1): no max pass); ctx^T = V_aug.T @ P accumulated over key blocks --
    rows 64:128 replicate the softmax denominator.  A fast full-tile copy
    releases the PSUM slot; normalize (reciprocal + multiply) runs in SBUF.
  - after each query chunk, a small AllToAll ships 64-token slivers of ctx^T
    to their owner cores, so the collectives ride under later compute.
  - batch-1 projections interleave into batch-0's attention; batch-0's
    output projection interleaves into batch-1's attention.  The serial
    tail is one sliver AllToAll + the last 128-token projection block.
"""

import os
import sys

import numpy as np

for _p in ("/opt/trn_rl_repo",):
    if os.path.isdir(_p) and _p not in sys.path:
        sys.path.append(_p)

import ml_dtypes

B, S, D, H = 2, 2048, 1024, 16
HD = D // H          # 64
NCORES = 8
T = B * S            # 4096 flattened tokens
DC = D // NCORES     # 128 head-dim columns per core (2 heads)
P = 128
KT_TILES = D // P    # 8 contraction tiles over d_model
SB_Q = S // 512      # 4 query chunks per batch
KB = S // P          # 16 key blocks per batch
NTB = T // P         # 32 token blocks
SLIV = 512 // NCORES  # 64-token sliver per (qc, dest core)
CH = SB_Q * SLIV     # 256 tokens per core per batch

_bf = ml_dtypes.bfloat16

PROFILE = False
LAST_RESULTS = None

_CACHE = {}


def _build(with_bias_v, with_bias_o, with_bias_qk):
    import concourse.mybir as mybir
    import concourse.tile as tile
    from concourse import bacc
    from concourse.bass import ds, ts
    from contextlib import ExitStack

    f32 = mybir.dt.float32
    bf16 = mybir.dt.bfloat16
    DT = bf16
    AF = mybir.ActivationFunctionType

    nc = bacc.Bacc("TRN2", target_bir_lowering=False, debug=False,
                   num_devices=NCORES)

    xT = nc.dram_tensor("xT", [D, T], DT, kind="ExternalInput")
    wq = nc.dram_tensor("wq", [D, DC], DT, kind="ExternalInput")
    wk = nc.dram_tensor("wk", [D, DC], DT, kind="ExternalInput")
    wv = nc.dram_tensor("wv", [D, DC], DT, kind="ExternalInput")
    wo = nc.dram_tensor("wo", [D, D], DT, kind="ExternalInput")
    bqd = nc.dram_tensor("bqv", [DC, 1], f32, kind="ExternalInput")
    bkd = nc.dram_tensor("bkv", [DC, 1], f32, kind="ExternalInput")
    bvd = nc.dram_tensor("bvv", [1, DC], DT, kind="ExternalInput")
    bod = nc.dram_tensor("bov", [1, D], DT, kind="ExternalInput")
    out = nc.dram_tensor("out", [B * CH, D], f32, kind="ExternalOutput")

    with tile.TileContext(nc) as tc, ExitStack() as ctx:
        sb = ctx.enter_context(tc.tile_pool(name="persist", bufs=1))
        dram = ctx.enter_context(tc.tile_pool(name="dram", bufs=1, space="DRAM"))
        psum = ctx.enter_context(tc.tile_pool(name="psum", bufs=1, space="PSUM"))
        ptp = ctx.enter_context(tc.tile_pool(name="ptp", bufs=3))
        nrm = ctx.enter_context(tc.tile_pool(name="nrm", bufs=3))
        osb_p = ctx.enter_context(tc.tile_pool(name="osbp", bufs=3))

        xts = sb.tile([P, KT_TILES, T], DT)
        qt = [sb.tile([HD, T], DT, name=f"qt{h}") for h in range(2)]
        kt = [sb.tile([HD, T], DT, name=f"kt{h}") for h in range(2)]
        va = sb.tile([P, NTB, 2, P], DT)   # V_aug: cols 0:64 V, 64:128 ones
        wq_s = sb.tile([P, KT_TILES, DC], DT)
        wk_s = sb.tile([P, KT_TILES, DC], DT)
        wv_s = sb.tile([P, KT_TILES, DC], DT)
        wo_s = sb.tile([P, KT_TILES, D], DT)
        ctxt = [sb.tile([P, KT_TILES, CH], DT, name=f"ctxt{b}") for b in range(B)]
        ones = sb.tile([1, P], DT)
        bq_s = sb.tile([DC, 1], f32)
        bk_s = sb.tile([DC, 1], f32)
        bv_s = sb.tile([1, DC], DT)
        bo_s = sb.tile([1, D], DT)
        warm = sb.tile([1, 32], f32)

        nc.vector.memset(ones[:], 1.0)
        nc.vector.memset(va[:], 1.0)  # ones columns [.., 64:128] survive
        nc.vector.memset(warm[:], 0.0)
        nc.scalar.activation(warm[:], warm[:], AF.Exp, scale=1.0)

        if with_bias_qk:
            nc.sync.dma_start(out=bq_s[:], in_=bqd.ap())
            nc.sync.dma_start(out=bk_s[:], in_=bkd.ap())
        if with_bias_v:
            nc.sync.dma_start(out=bv_s[:], in_=bvd.ap())
        if with_bias_o:
            nc.sync.dma_start(out=bo_s[:], in_=bod.ap())

        # input DMAs: qkv weights first, then x^T (chunk 0 first), wo last
        nc.sync.dma_start(out=wq_s[:], in_=wq.ap().rearrange("(k p) c -> p k c", p=P))
        nc.sync.dma_start(out=wk_s[:], in_=wk.ap().rearrange("(k p) c -> p k c", p=P))
        nc.sync.dma_start(out=wv_s[:], in_=wv.ap().rearrange("(k p) c -> p k c", p=P))
        xT3 = xT.ap().rearrange("(k p) t -> k p t", p=P)
        for qcg in range(T // 512):
            for kti in range(KT_TILES):
                nc.sync.dma_start(out=xts[:, kti, ts(qcg, 512)],
                                  in_=xT3[kti][:, ts(qcg, 512)])
        wo3 = wo.ap().rearrange("(k p) e -> k p e", p=P)
        for kti in range(KT_TILES):
            nc.sync.dma_start(out=wo_s[:, kti], in_=wo3[kti])

        # per-(batch, qc) sliver AllToAll buffers: [dest, 128 d-rows, 64 tok]
        a2a_in = [[dram.tile([NCORES, P, SLIV], DT, name=f"a2ai{b}_{qc}")
                   for qc in range(SB_Q)] for b in range(B)]
        a2a_out = [[dram.tile([NCORES, P, SLIV], DT, name=f"a2ao{b}_{qc}")
                    for qc in range(SB_Q)] for b in range(B)]

        def proj_qk(qcg, w_s, b_s, dsts, wb, tag):
            ps = psum.tile([P, 512], f32, tag=tag,
                           bufs=(3 if tag == "ctx" else 1), name=f"pqk{qcg}")
            for kti in range(KT_TILES):
                nc.tensor.matmul(ps[:], w_s[:, kti], xts[:, kti, ts(qcg, 512)],
                                 start=(kti == 0), stop=(kti == KT_TILES - 1))
            for h in range(2):
                sl = ps[h * HD:(h + 1) * HD, :]
                if wb:
                    nc.scalar.activation(dsts[h][:, ts(qcg, 512)], sl,
                                         AF.Relu, bias=b_s[h * HD:(h + 1) * HD, :])
                else:
                    nc.vector.tensor_scalar_max(dsts[h][:, ts(qcg, 512)], sl, 0.0)

        def proj_v(tb, tag):
            vps = psum.tile([P, DC], f32, tag=tag,
                            bufs=(3 if tag == "ctx" else 1), name=f"pv{tb}")
            if with_bias_v:
                nc.tensor.matmul(vps[:], ones[:], bv_s[:], start=True, stop=False)
            for kti in range(KT_TILES):
                nc.tensor.matmul(vps[:], xts[:, kti, ts(tb, P)], wv_s[:, kti],
                                 start=(kti == 0 and not with_bias_v),
                                 stop=(kti == KT_TILES - 1))
            for h in range(2):
                nc.vector.tensor_scalar_max(va[:, tb, h, 0:HD],
                                            vps[:, h * HD:(h + 1) * HD], 0.0)

        # gather one qc's sliver exchange into ctxt[b]
        def gather(b, qc):
            for i in range(NCORES):
                nc.sync.dma_start(out=ctxt[b][:, i, ts(qc, SLIV)],
                                  in_=a2a_out[b][qc][i])

        # output projection for one 128-token block of this core's share
        def outproj_block(b, tb, tag="proj"):
            for ec in range(D // 512):
                ps = psum.tile([P, 512], f32, tag=tag,
                               bufs=(3 if tag == "ctx" else 1),
                               name=f"po{b}_{tb}_{ec}")
                if with_bias_o:
                    nc.tensor.matmul(ps[:], ones[:], bo_s[:, ts(ec, 512)],
                                     start=True, stop=False)
                for kti in range(KT_TILES):
                    nc.tensor.matmul(ps[:], ctxt[b][:, kti, ts(tb, P)],
                                     wo_s[:, kti, ts(ec, 512)],
                                     start=(kti == 0 and not with_bias_o),
                                     stop=(kti == KT_TILES - 1))
                osb = osb_p.tile([P, 512], f32, tag="osb")
                nc.vector.tensor_scalar_max(osb[:], ps[:], 0.0)
                nc.sync.dma_start(out=out.ap()[ds(b * CH + tb * P, P), ts(ec, 512)],
                                  in_=osb[:])

        # attention for one batch; fillers[i] emitted at fractional positions
        def attention(b, fillers, positions):
            order = sorted(range(len(fillers)), key=lambda i: positions[i])
            fi = 0
            n_iter = SB_Q * KB
            it = 0
            for qc in range(SB_Q):
                qsl = ds(b * S + qc * 512, 512)
                cps = [psum.tile([P, 512], f32, tag="ctx", bufs=3,
                                 name=f"cps{b}_{qc}_{h}") for h in range(2)]
                for kb in range(KB):
                    ksl = ds(b * S + kb * P, P)
                    sps = psum.tile([P, 2, 512], f32, tag="sc", bufs=2)
                    for h in range(2):
                        nc.tensor.matmul(sps[:, h], kt[h][:, ksl], qt[h][:, qsl],
                                         start=True, stop=True)
                    pt = ptp.tile([P, 2, 512], DT, tag="p")
                    nc.scalar.activation(pt[:], sps[:], AF.Exp, scale=0.125)
                    for h in range(2):
                        nc.tensor.matmul(cps[h][:], va[:, b * KB + kb, h], pt[:, h],
                                         start=(kb == 0), stop=(kb == KB - 1))
                    it += 1
                    while fi < len(order) and positions[order[fi]] * n_iter < it:
                        fillers[order[fi]]()
                        fi += 1
                # normalize in SBUF after a fast PSUM-releasing copy
                for h in range(2):
                    cfull = nrm.tile([P, 512], f32, tag="cf")
                    nc.vector.tensor_copy(cfull[:], cps[h][:])
                    recb = nrm.tile([HD, 512], f32, tag="recb")
                    nc.vector.reciprocal(recb[:], cfull[HD:P, :])
                    csb = nrm.tile([HD, 512], DT, tag="csb")
                    nc.vector.tensor_tensor(csb[:], cfull[0:HD, :], recb[:],
                                            mybir.AluOpType.mult)
                    # one strided DMA scatters the 8 64-token slivers
                    nc.sync.dma_start(
                        out=a2a_in[b][qc][:, h * HD:(h + 1) * HD, :]
                            .rearrange("j p c -> p j c"),
                        in_=csb[:].rearrange("p (j c) -> p j c", j=NCORES))
                nc.gpsimd.collective_compute(
                    "AllToAll", mybir.AluOpType.bypass,
                    replica_groups=[list(range(NCORES))],
                    ins=[a2a_in[b][qc].opt()], outs=[a2a_out[b][qc].opt()],
                )
            for i in order[fi:]:
                fillers[i]()

        # ================= schedule =================
        for qcg in range(SB_Q):
            proj_qk(qcg, wq_s, bq_s, qt, with_bias_qk, tag="ctx")
            proj_qk(qcg, wk_s, bk_s, kt, with_bias_qk, tag="ctx")
            for tb in range(4 * qcg, 4 * qcg + 4):
                proj_v(tb, tag="ctx")

        # batch-0 attention with batch-1 projections as filler
        fillers, pos = [], []
        for qcg in range(SB_Q, 2 * SB_Q):
            fillers.append(lambda q=qcg: proj_qk(q, wq_s, bq_s, qt, with_bias_qk, "proj"))
            fillers.append(lambda q=qcg: proj_qk(q, wk_s, bk_s, kt, with_bias_qk, "proj"))
            for tb in range(4 * qcg, 4 * qcg + 4):
                fillers.append(lambda t=tb: proj_v(t, "proj"))
        pos = [(i + 0.5) / len(fillers) for i in range(len(fillers))]
        attention(0, fillers, pos)

        # batch-1 attention; fillers: gather batch-0 slivers + project batch-0
        fillers = [lambda q=qc: gather(0, q) for qc in range(SB_Q)]
        pos = [0.06 + 0.03 * qc for qc in range(SB_Q)]
        fillers.append(lambda: outproj_block(0, 0))
        pos.append(0.30)
        fillers.append(lambda: outproj_block(0, 1))
        pos.append(0.60)
        attention(1, fillers, pos)

        # tail: gather batch-1 slivers (qc 0-1 landed mid-attention) + project
        gather(1, 0)
        gather(1, 1)
        outproj_block(1, 0, tag="ctx")
        gather(1, 2)
        gather(1, 3)
        outproj_block(1, 1, tag="ctx")

    nc.compile()
    return nc


def _get(with_bias_v, with_bias_o, with_bias_qk):
    key = (with_bias_v, with_bias_o, with_bias_qk)
    if key not in _CACHE:
        _CACHE[key] = _build(*key)
    return _CACHE[key]


def kernel(x, Wq, bq, Wk, bk, Wv, bv, Wo, bo):
    global LAST_RESULTS
    from concourse.bass_utils import run_bass_kernel_spmd

    x = np.asarray(x, dtype=np.float32)
    Wq, Wk, Wv, Wo = (np.asarray(w, dtype=np.float32) for w in (Wq, Wk, Wv, Wo))
    bq, bk, bv, bo = (np.asarray(v, dtype=np.float32) for v in (bq, bk, bv, bo))

    wb_qk = bool(np.any(bq) or np.any(bk))
    wb_v = bool(np.any(bv))
    wb_o = bool(np.any(bo))
    nc = _get(wb_v, wb_o, wb_qk)

    xT = np.ascontiguousarray(x.reshape(T, D).astype(_bf).T)
    Wq16 = Wq.astype(_bf)
    Wk16 = Wk.astype(_bf)
    Wv16 = Wv.astype(_bf)
    Wo16 = np.ascontiguousarray(Wo.astype(_bf))
    bv16 = bv.astype(_bf)
    bo16 = np.ascontiguousarray(bo.astype(_bf).reshape(1, D))

    in_maps = []
    for c in range(NCORES):
        cs = slice(c * DC, (c + 1) * DC)
        in_maps.append({
            "xT": xT,
            "wq": np.ascontiguousarray(Wq16[:, cs]),
            "wk": np.ascontiguousarray(Wk16[:, cs]),
            "wv": np.ascontiguousarray(Wv16[:, cs]),
            "wo": Wo16,
            "bqv": np.ascontiguousarray(bq[cs].reshape(DC, 1)),
            "bkv": np.ascontiguousarray(bk[cs].reshape(DC, 1)),
            "bvv": np.ascontiguousarray(bv16[cs].reshape(1, DC)),
            "bov": bo16,
        })

    kw = {}
    if PROFILE:
        kw = dict(trace=True, trace_cores=[0])
    res = run_bass_kernel_spmd(nc, in_maps, core_ids=list(range(NCORES)), **kw)
    LAST_RESULTS = res

    # core j's out rows (b*256 + qc*64 + c) hold tokens (b, qc*512 + j*64 + c)
    full = np.empty((B, SB_Q, NCORES, SLIV, D), np.float32)
    for j in range(NCORES):
        full[:, :, j] = res.results[j]["out"].reshape(B, SB_Q, SLIV, D)
    return np.ascontiguousarray(full.reshape(B, S, D))


# revision 9
# speedup vs baseline: 1.0779x; 1.0779x over previous
"""Multi-head attention forward on 8 Trainium2 NeuronCores.

Problem (all shapes hardcoded): B=2, S=2048, D=1024, H=16, HD=64
    q = relu(x @ Wq + bq); k = relu(x @ Wk + bk); v = relu(x @ Wv + bv)
    attn = softmax(q k^T / sqrt(HD)) per (batch, head)
    out = relu((attn @ v) @ Wo + bo)

Sharding: head-parallel for QKV+attention (2 heads per core, both batches),
then AllToAlls re-shard the per-head context to a per-token shard and each
core runs the full output projection for its 512 tokens. Host reassembles.

Device schedule (per core):
  - Q^T, K^T ([64, 4096] per head) and V_aug ([128 tokens, 64 V cols + 64
    ones cols] per head block) via bf16 matmuls against x^T, fp32 PSUM.
  - scores computed transposed S^T[k, q] = K^T.T @ Q^T per head (K_c=64);
    exp on ACT straight from PSUM with the 1/8 scale folded in (scores are
    O(1): no max pass); ctx^T = V_aug.T @ P accumulated over key blocks --
    rows 64:128 replicate the softmax denominator.  A fast full-tile copy
    releases the PSUM slot; normalize (reciprocal + multiply) runs in SBUF.
  - after each query chunk, a small AllToAll ships 64-token slivers of ctx^T
    to their owner cores, so the collectives ride under later compute.
  - batch-1 projections interleave into batch-0's attention; batch-0's
    output projection interleaves into batch-1's attention.  The serial
    tail is one sliver AllToAll + the last 128-token projection block.
"""

import os
import sys

import numpy as np

for _p in ("/opt/trn_rl_repo",):
    if os.path.isdir(_p) and _p not in sys.path:
        sys.path.append(_p)

import ml_dtypes

B, S, D, H = 2, 2048, 1024, 16
HD = D // H          # 64
NCORES = 8
T = B * S            # 4096 flattened tokens
DC = D // NCORES     # 128 head-dim columns per core (2 heads)
P = 128
KT_TILES = D // P    # 8 contraction tiles over d_model
SB_Q = S // 512      # 4 query chunks per batch
KB = S // P          # 16 key blocks per batch
NTB = T // P         # 32 token blocks
SLIV = 512 // NCORES  # 64-token sliver per (qc, dest core)
CH = SB_Q * SLIV     # 256 tokens per core per batch

_bf = ml_dtypes.bfloat16

PROFILE = False
LAST_RESULTS = None

_CACHE = {}


def _build(with_bias_v, with_bias_o, with_bias_qk):
    import concourse.mybir as mybir
    import concourse.tile as tile
    from concourse import bacc
    from concourse.bass import ds, ts
    from contextlib import ExitStack

    f32 = mybir.dt.float32
    bf16 = mybir.dt.bfloat16
    DT = bf16
    AF = mybir.ActivationFunctionType

    nc = bacc.Bacc("TRN2", target_bir_lowering=False, debug=False,
                   num_devices=NCORES)

    xT = nc.dram_tensor("xT", [D, T], DT, kind="ExternalInput")
    wq = nc.dram_tensor("wq", [D, DC], DT, kind="ExternalInput")
    wk = nc.dram_tensor("wk", [D, DC], DT, kind="ExternalInput")
    wv = nc.dram_tensor("wv", [D, DC], DT, kind="ExternalInput")
    wo = nc.dram_tensor("wo", [D, D], DT, kind="ExternalInput")
    bqd = nc.dram_tensor("bqv", [DC, 1], f32, kind="ExternalInput")
    bkd = nc.dram_tensor("bkv", [DC, 1], f32, kind="ExternalInput")
    bvd = nc.dram_tensor("bvv", [1, DC], DT, kind="ExternalInput")
    bod = nc.dram_tensor("bov", [1, D], DT, kind="ExternalInput")
    out = nc.dram_tensor("out", [B * CH, D], f32, kind="ExternalOutput")

    with tile.TileContext(nc) as tc, ExitStack() as ctx:
        sb = ctx.enter_context(tc.tile_pool(name="persist", bufs=1))
        dram = ctx.enter_context(tc.tile_pool(name="dram", bufs=1, space="DRAM"))
        psum = ctx.enter_context(tc.tile_pool(name="psum", bufs=1, space="PSUM"))
        ptp = ctx.enter_context(tc.tile_pool(name="ptp", bufs=3))
        nrm = ctx.enter_context(tc.tile_pool(name="nrm", bufs=3))
        osb_p = ctx.enter_context(tc.tile_pool(name="osbp", bufs=3))

        xts = sb.tile([P, KT_TILES, T], DT)
        qt = [sb.tile([HD, T], DT, name=f"qt{h}") for h in range(2)]
        kt = [sb.tile([HD, T], DT, name=f"kt{h}") for h in range(2)]
        va = sb.tile([P, NTB, 2, P], DT)   # V_aug: cols 0:64 V, 64:128 ones
        wq_s = sb.tile([P, KT_TILES, DC], DT)
        wk_s = sb.tile([P, KT_TILES, DC], DT)
        wv_s = sb.tile([P, KT_TILES, DC], DT)
        wo_s = sb.tile([P, KT_TILES, D], DT)
        ctxt = [sb.tile([P, KT_TILES, CH], DT, name=f"ctxt{b}") for b in range(B)]
        ones = sb.tile([1, P], DT)
        bq_s = sb.tile([DC, 1], f32)
        bk_s = sb.tile([DC, 1], f32)
        bv_s = sb.tile([1, DC], DT)
        bo_s = sb.tile([1, D], DT)
        warm = sb.tile([1, 32], f32)

        nc.vector.memset(ones[:], 1.0)
        nc.vector.memset(va[:], 1.0)  # ones columns [.., 64:128] survive
        nc.vector.memset(warm[:], 0.0)
        nc.scalar.activation(warm[:], warm[:], AF.Exp, scale=1.0)

        if with_bias_qk:
            nc.sync.dma_start(out=bq_s[:], in_=bqd.ap())
            nc.sync.dma_start(out=bk_s[:], in_=bkd.ap())
        if with_bias_v:
            nc.sync.dma_start(out=bv_s[:], in_=bvd.ap())
        if with_bias_o:
            nc.sync.dma_start(out=bo_s[:], in_=bod.ap())

        # input DMAs: qkv weights first, then x^T (chunk 0 first), wo last
        nc.sync.dma_start(out=wq_s[:], in_=wq.ap().rearrange("(k p) c -> p k c", p=P))
        nc.sync.dma_start(out=wk_s[:], in_=wk.ap().rearrange("(k p) c -> p k c", p=P))
        nc.sync.dma_start(out=wv_s[:], in_=wv.ap().rearrange("(k p) c -> p k c", p=P))
        xT3 = xT.ap().rearrange("(k p) t -> k p t", p=P)
        for qcg in range(T // 512):
            for kti in range(KT_TILES):
                nc.sync.dma_start(out=xts[:, kti, ts(qcg, 512)],
                                  in_=xT3[kti][:, ts(qcg, 512)])
        wo3 = wo.ap().rearrange("(k p) e -> k p e", p=P)
        for kti in range(KT_TILES):
            nc.sync.dma_start(out=wo_s[:, kti], in_=wo3[kti])

        # per-batch AllToAll buffers: [dest core, 128 d-rows, 256 tokens]
        a2a_in = [dram.tile([NCORES, P, CH], DT, name=f"a2ai{b}") for b in range(B)]
        a2a_out = [dram.tile([NCORES, P, CH], DT, name=f"a2ao{b}") for b in range(B)]
        # tiny warm-up collective: absorbs the first-call ncfw/descriptor
        # staging latency during the projection phase
        wcc_in = dram.tile([NCORES, 16, 16], DT)
        wcc_out = dram.tile([NCORES, 16, 16], DT)
        wcc_sb = sb.tile([16, NCORES * 16], DT)
        nc.vector.memset(wcc_sb[:], 0.0)
        nc.sync.dma_start(out=wcc_in[:].rearrange("j p c -> p j c"),
                          in_=wcc_sb[:].rearrange("p (j c) -> p j c", j=NCORES))
        nc.gpsimd.collective_compute(
            "AllToAll", mybir.AluOpType.bypass,
            replica_groups=[list(range(NCORES))],
            ins=[wcc_in.opt()], outs=[wcc_out.opt()],
        )

        def proj_qk(qcg, w_s, b_s, dsts, wb, tag):
            ps = psum.tile([P, 512], f32, tag=tag,
                           bufs=(3 if tag == "ctx" else 1), name=f"pqk{qcg}")
            for kti in range(KT_TILES):
                nc.tensor.matmul(ps[:], w_s[:, kti], xts[:, kti, ts(qcg, 512)],
                                 start=(kti == 0), stop=(kti == KT_TILES - 1))
            for h in range(2):
                sl = ps[h * HD:(h + 1) * HD, :]
                if wb:
                    nc.scalar.activation(dsts[h][:, ts(qcg, 512)], sl,
                                         AF.Relu, bias=b_s[h * HD:(h + 1) * HD, :])
                else:
                    nc.vector.tensor_scalar_max(dsts[h][:, ts(qcg, 512)], sl, 0.0)

        def proj_v(tb, tag):
            vps = psum.tile([P, DC], f32, tag=tag,
                            bufs=(3 if tag == "ctx" else 1), name=f"pv{tb}")
            if with_bias_v:
                nc.tensor.matmul(vps[:], ones[:], bv_s[:], start=True, stop=False)
            for kti in range(KT_TILES):
                nc.tensor.matmul(vps[:], xts[:, kti, ts(tb, P)], wv_s[:, kti],
                                 start=(kti == 0 and not with_bias_v),
                                 stop=(kti == KT_TILES - 1))
            for h in range(2):
                nc.vector.tensor_scalar_max(va[:, tb, h, 0:HD],
                                            vps[:, h * HD:(h + 1) * HD], 0.0)

        # gather one qc's sliver exchange into ctxt[b]
        def gather(b):
            for i in range(NCORES):
                nc.sync.dma_start(out=ctxt[b][:, i, :], in_=a2a_out[b][i])

        # output projection for one 128-token block of this core's share
        def outproj_block(b, tb, tag="proj"):
            for ec in range(D // 512):
                ps = psum.tile([P, 512], f32, tag=tag,
                               bufs=(3 if tag == "ctx" else 1),
                               name=f"po{b}_{tb}_{ec}")
                if with_bias_o:
                    nc.tensor.matmul(ps[:], ones[:], bo_s[:, ts(ec, 512)],
                                     start=True, stop=False)
                for kti in range(KT_TILES):
                    nc.tensor.matmul(ps[:], ctxt[b][:, kti, ts(tb, P)],
                                     wo_s[:, kti, ts(ec, 512)],
                                     start=(kti == 0 and not with_bias_o),
                                     stop=(kti == KT_TILES - 1))
                osb = osb_p.tile([P, 512], f32, tag="osb")
                nc.vector.tensor_scalar_max(osb[:], ps[:], 0.0)
                nc.sync.dma_start(out=out.ap()[ds(b * CH + tb * P, P), ts(ec, 512)],
                                  in_=osb[:])

        # attention for one batch; fillers[i] emitted at fractional positions
        def attention(b, fillers, positions):
            order = sorted(range(len(fillers)), key=lambda i: positions[i])
            fi = 0
            n_iter = SB_Q * KB
            it = 0
            for qc in range(SB_Q):
                qsl = ds(b * S + qc * 512, 512)
                cps = [psum.tile([P, 512], f32, tag="ctx", bufs=3,
                                 name=f"cps{b}_{qc}_{h}") for h in range(2)]
                for kb in range(KB):
                    ksl = ds(b * S + kb * P, P)
                    sps = psum.tile([P, 2, 512], f32, tag="sc", bufs=2)
                    for h in range(2):
                        nc.tensor.matmul(sps[:, h], kt[h][:, ksl], qt[h][:, qsl],
                                         start=True, stop=True)
                    pt = ptp.tile([P, 2, 512], DT, tag="p")
                    nc.scalar.activation(pt[:], sps[:], AF.Exp, scale=0.125)
                    for h in range(2):
                        nc.tensor.matmul(cps[h][:], va[:, b * KB + kb, h], pt[:, h],
                                         start=(kb == 0), stop=(kb == KB - 1))
                    it += 1
                    while fi < len(order) and positions[order[fi]] * n_iter < it:
                        fillers[order[fi]]()
                        fi += 1
                # normalize in SBUF after a fast PSUM-releasing copy
                for h in range(2):
                    cfull = nrm.tile([P, 512], f32, tag="cf")
                    nc.vector.tensor_copy(cfull[:], cps[h][:])
                    recb = nrm.tile([HD, 512], f32, tag="recb")
                    nc.vector.reciprocal(recb[:], cfull[HD:P, :])
                    csb = nrm.tile([HD, 512], DT, tag="csb")
                    nc.vector.tensor_tensor(csb[:], cfull[0:HD, :], recb[:],
                                            mybir.AluOpType.mult)
                    # scatter the two 256-token half-chunks (dest cores 2qc, 2qc+1)
                    nc.sync.dma_start(
                        out=a2a_in[b][2 * qc:2 * qc + 2, h * HD:(h + 1) * HD, :]
                            .rearrange("j p c -> p j c"),
                        in_=csb[:].rearrange("p (j c) -> p j c", j=2))
            nc.gpsimd.collective_compute(
                "AllToAll", mybir.AluOpType.bypass,
                replica_groups=[list(range(NCORES))],
                ins=[a2a_in[b].opt()], outs=[a2a_out[b].opt()],
            )
            for i in order[fi:]:
                fillers[i]()

        # ================= schedule =================
        for qcg in range(SB_Q):
            proj_qk(qcg, wq_s, bq_s, qt, with_bias_qk, tag="ctx")
            proj_qk(qcg, wk_s, bk_s, kt, with_bias_qk, tag="ctx")
            for tb in range(4 * qcg, 4 * qcg + 4):
                proj_v(tb, tag="ctx")

        # batch-0 attention with batch-1 projections as filler
        fillers, pos = [], []
        for qcg in range(SB_Q, 2 * SB_Q):
            fillers.append(lambda q=qcg: proj_qk(q, wq_s, bq_s, qt, with_bias_qk, "proj"))
            fillers.append(lambda q=qcg: proj_qk(q, wk_s, bk_s, kt, with_bias_qk, "proj"))
            for tb in range(4 * qcg, 4 * qcg + 4):
                fillers.append(lambda t=tb: proj_v(t, "proj"))
        pos = [(i + 0.5) / len(fillers) for i in range(len(fillers))]
        attention(0, fillers, pos)

        # batch-1 attention; fillers: gather batch-0 ctx + project batch-0
        fillers = [lambda: gather(0),
                   lambda: outproj_block(0, 0),
                   lambda: outproj_block(0, 1)]
        pos = [0.30, 0.40, 0.70]
        attention(1, fillers, pos)

        # tail: gather batch-1 ctx + project its two token blocks
        gather(1)
        outproj_block(1, 0, tag="ctx")
        outproj_block(1, 1, tag="ctx")

    nc.compile()
    return nc


def _get(with_bias_v, with_bias_o, with_bias_qk):
    key = (with_bias_v, with_bias_o, with_bias_qk)
    if key not in _CACHE:
        _CACHE[key] = _build(*key)
    return _CACHE[key]


def kernel(x, Wq, bq, Wk, bk, Wv, bv, Wo, bo):
    global LAST_RESULTS
    from concourse.bass_utils import run_bass_kernel_spmd

    x = np.asarray(x, dtype=np.float32)
    Wq, Wk, Wv, Wo = (np.asarray(w, dtype=np.float32) for w in (Wq, Wk, Wv, Wo))
    bq, bk, bv, bo = (np.asarray(v, dtype=np.float32) for v in (bq, bk, bv, bo))

    wb_qk = bool(np.any(bq) or np.any(bk))
    wb_v = bool(np.any(bv))
    wb_o = bool(np.any(bo))
    nc = _get(wb_v, wb_o, wb_qk)

    xT = np.ascontiguousarray(x.reshape(T, D).astype(_bf).T)
    Wq16 = Wq.astype(_bf)
    Wk16 = Wk.astype(_bf)
    Wv16 = Wv.astype(_bf)
    Wo16 = np.ascontiguousarray(Wo.astype(_bf))
    bv16 = bv.astype(_bf)
    bo16 = np.ascontiguousarray(bo.astype(_bf).reshape(1, D))

    in_maps = []
    for c in range(NCORES):
        cs = slice(c * DC, (c + 1) * DC)
        in_maps.append({
            "xT": xT,
            "wq": np.ascontiguousarray(Wq16[:, cs]),
            "wk": np.ascontiguousarray(Wk16[:, cs]),
            "wv": np.ascontiguousarray(Wv16[:, cs]),
            "wo": Wo16,
            "bqv": np.ascontiguousarray(bq[cs].reshape(DC, 1)),
            "bkv": np.ascontiguousarray(bk[cs].reshape(DC, 1)),
            "bvv": np.ascontiguousarray(bv16[cs].reshape(1, DC)),
            "bov": bo16,
        })

    kw = {}
    if PROFILE:
        kw = dict(trace=True, trace_cores=[0])
    res = run_bass_kernel_spmd(nc, in_maps, core_ids=list(range(NCORES)), **kw)
    LAST_RESULTS = res

    # core j's out rows: 0:CH -> batch-0 tokens [j*CH, (j+1)*CH), CH:2CH ->
    # batch-1 tokens [S + j*CH, ...)
    full = np.empty((T, D), np.float32)
    for j in range(NCORES):
        o = res.results[j]["out"]
        full[j * CH:(j + 1) * CH] = o[0:CH]
        full[S + j * CH:S + (j + 1) * CH] = o[CH:2 * CH]
    return np.ascontiguousarray(full.reshape(B, S, D))


# revision 11
# speedup vs baseline: 1.1358x; 1.0537x over previous
"""Multi-head attention forward on 8 Trainium2 NeuronCores.

Problem (all shapes hardcoded): B=2, S=2048, D=1024, H=16, HD=64
    q = relu(x @ Wq + bq); k = relu(x @ Wk + bk); v = relu(x @ Wv + bv)
    attn = softmax(q k^T / sqrt(HD)) per (batch, head)
    out = relu((attn @ v) @ Wo + bo)

Sharding: head-parallel for QKV+attention (2 heads per core, both batches),
then AllToAlls re-shard the per-head context to a per-token shard and each
core runs the full output projection for its 512 tokens. Host reassembles.

Device schedule (per core):
  - Q^T, K^T ([64, 4096] per head) and V_aug ([128 tokens, 64 V cols + 64
    ones cols] per head block) via bf16 matmuls against x^T, fp32 PSUM.
  - scores computed transposed S^T[k, q] = K^T.T @ Q^T per head (K_c=64);
    exp on ACT straight from PSUM with the 1/8 scale folded in (scores are
    O(1): no max pass); ctx^T = V_aug.T @ P accumulated over key blocks --
    rows 64:128 replicate the softmax denominator.  A fast full-tile copy
    releases the PSUM slot; normalize (reciprocal + multiply) runs in SBUF.
  - after each query chunk, a small AllToAll ships 64-token slivers of ctx^T
    to their owner cores, so the collectives ride under later compute.
  - batch-1 projections interleave into batch-0's attention; batch-0's
    output projection interleaves into batch-1's attention.  The serial
    tail is one sliver AllToAll + the last 128-token projection block.
"""

import os
import sys

import numpy as np

for _p in ("/opt/trn_rl_repo",):
    if os.path.isdir(_p) and _p not in sys.path:
        sys.path.append(_p)

import ml_dtypes

B, S, D, H = 2, 2048, 1024, 16
HD = D // H          # 64
NCORES = 8
T = B * S            # 4096 flattened tokens
DC = D // NCORES     # 128 head-dim columns per core (2 heads)
P = 128
KT_TILES = D // P    # 8 contraction tiles over d_model
SB_Q = S // 512      # 4 query chunks per batch
KB = S // P          # 16 key blocks per batch
NTB = T // P         # 32 token blocks
SLIV = 512 // NCORES  # 64-token sliver per (qc, dest core)
CH = SB_Q * SLIV     # 256 tokens per core per batch

_bf = ml_dtypes.bfloat16

PROFILE = False
LAST_RESULTS = None

_CACHE = {}


def _build(with_bias_v, with_bias_o, with_bias_qk):
    import concourse.mybir as mybir
    import concourse.tile as tile
    from concourse import bacc
    from concourse.bass import ds, ts
    from contextlib import ExitStack

    f32 = mybir.dt.float32
    bf16 = mybir.dt.bfloat16
    DT = bf16
    AF = mybir.ActivationFunctionType

    nc = bacc.Bacc("TRN2", target_bir_lowering=False, debug=False,
                   num_devices=NCORES)

    xT = nc.dram_tensor("xT", [D, T], DT, kind="ExternalInput")
    wq = nc.dram_tensor("wq", [D, DC], DT, kind="ExternalInput")
    wk = nc.dram_tensor("wk", [D, DC], DT, kind="ExternalInput")
    wv = nc.dram_tensor("wv", [D, DC], DT, kind="ExternalInput")
    wo = nc.dram_tensor("wo", [D, D], DT, kind="ExternalInput")
    bqd = nc.dram_tensor("bqv", [DC, 1], f32, kind="ExternalInput")
    bkd = nc.dram_tensor("bkv", [DC, 1], f32, kind="ExternalInput")
    bvd = nc.dram_tensor("bvv", [1, DC], DT, kind="ExternalInput")
    bod = nc.dram_tensor("bov", [1, D], DT, kind="ExternalInput")
    out = nc.dram_tensor("out", [B * CH, D], f32, kind="ExternalOutput")

    with tile.TileContext(nc) as tc, ExitStack() as ctx:
        sb = ctx.enter_context(tc.tile_pool(name="persist", bufs=1))
        dram = ctx.enter_context(tc.tile_pool(name="dram", bufs=1, space="DRAM"))
        psum = ctx.enter_context(tc.tile_pool(name="psum", bufs=1, space="PSUM"))
        ptp = ctx.enter_context(tc.tile_pool(name="ptp", bufs=3))
        nrm = ctx.enter_context(tc.tile_pool(name="nrm", bufs=3))
        osb_p = ctx.enter_context(tc.tile_pool(name="osbp", bufs=3))

        xts = sb.tile([P, KT_TILES, T], DT)
        qt = [sb.tile([HD, T], DT, name=f"qt{h}") for h in range(2)]
        kt = [sb.tile([HD, T], DT, name=f"kt{h}") for h in range(2)]
        va = sb.tile([P, NTB, 2, P], DT)   # V_aug: cols 0:64 V, 64:128 ones
        wq_s = sb.tile([P, KT_TILES, DC], DT)
        wk_s = sb.tile([P, KT_TILES, DC], DT)
        wv_s = sb.tile([P, KT_TILES, DC], DT)
        wo_s = sb.tile([P, KT_TILES, D], DT)
        ctxt = [sb.tile([P, KT_TILES, CH], DT, name=f"ctxt{b}") for b in range(B)]
        ones = sb.tile([1, P], DT)
        bq_s = sb.tile([DC, 1], f32)
        bk_s = sb.tile([DC, 1], f32)
        bv_s = sb.tile([1, DC], DT)
        bo_s = sb.tile([1, D], DT)
        warm = sb.tile([1, 32], f32)

        nc.vector.memset(ones[:], 1.0)
        nc.vector.memset(va[:], 1.0)  # ones columns [.., 64:128] survive
        nc.vector.memset(warm[:], 0.0)
        nc.scalar.activation(warm[:], warm[:], AF.Exp, scale=1.0)

        if with_bias_qk:
            nc.sync.dma_start(out=bq_s[:], in_=bqd.ap())
            nc.sync.dma_start(out=bk_s[:], in_=bkd.ap())
        if with_bias_v:
            nc.sync.dma_start(out=bv_s[:], in_=bvd.ap())
        if with_bias_o:
            nc.sync.dma_start(out=bo_s[:], in_=bod.ap())

        # input DMAs: qkv weights first, then x^T (chunk 0 first), wo last
        nc.sync.dma_start(out=wq_s[:], in_=wq.ap().rearrange("(k p) c -> p k c", p=P))
        nc.sync.dma_start(out=wk_s[:], in_=wk.ap().rearrange("(k p) c -> p k c", p=P))
        nc.sync.dma_start(out=wv_s[:], in_=wv.ap().rearrange("(k p) c -> p k c", p=P))
        xT3 = xT.ap().rearrange("(k p) t -> k p t", p=P)
        for qcg in range(T // 512):
            for kti in range(KT_TILES):
                nc.sync.dma_start(out=xts[:, kti, ts(qcg, 512)],
                                  in_=xT3[kti][:, ts(qcg, 512)])
        wo3 = wo.ap().rearrange("(k p) e -> k p e", p=P)
        for kti in range(KT_TILES):
            nc.sync.dma_start(out=wo_s[:, kti], in_=wo3[kti])

        # per-batch AllToAll buffers: [dest core, 128 d-rows, 256 tokens]
        a2a_in = [dram.tile([NCORES, P, CH], DT, name=f"a2ai{b}") for b in range(B)]
        a2a_out = [dram.tile([NCORES, P, CH], DT, name=f"a2ao{b}") for b in range(B)]
        # tiny warm-up collective: absorbs the first-call ncfw/descriptor
        # staging latency during the projection phase
        wcc_in = dram.tile([NCORES, 16, 16], DT)
        wcc_out = dram.tile([NCORES, 16, 16], DT)
        wcc_sb = sb.tile([16, NCORES * 16], DT)
        nc.vector.memset(wcc_sb[:], 0.0)
        nc.sync.dma_start(out=wcc_in[:].rearrange("j p c -> p j c"),
                          in_=wcc_sb[:].rearrange("p (j c) -> p j c", j=NCORES))
        nc.gpsimd.collective_compute(
            "AllToAll", mybir.AluOpType.bypass,
            replica_groups=[list(range(NCORES))],
            ins=[wcc_in.opt()], outs=[wcc_out.opt()],
        )

        def proj_qk(qcg, w_s, b_s, dsts, wb, tag):
            ps = psum.tile([P, 512], f32, tag=tag,
                           bufs=(3 if tag == "ctx" else 1), name=f"pqk{qcg}")
            for kti in range(KT_TILES):
                nc.tensor.matmul(ps[:], w_s[:, kti], xts[:, kti, ts(qcg, 512)],
                                 start=(kti == 0), stop=(kti == KT_TILES - 1))
            for h in range(2):
                sl = ps[h * HD:(h + 1) * HD, :]
                if wb:
                    nc.scalar.activation(dsts[h][:, ts(qcg, 512)], sl,
                                         AF.Relu, bias=b_s[h * HD:(h + 1) * HD, :])
                else:
                    nc.vector.tensor_scalar_max(dsts[h][:, ts(qcg, 512)], sl, 0.0)

        def proj_v(tb, tag):
            vps = psum.tile([P, DC], f32, tag=tag,
                            bufs=(3 if tag == "ctx" else 1), name=f"pv{tb}")
            if with_bias_v:
                nc.tensor.matmul(vps[:], ones[:], bv_s[:], start=True, stop=False)
            for kti in range(KT_TILES):
                nc.tensor.matmul(vps[:], xts[:, kti, ts(tb, P)], wv_s[:, kti],
                                 start=(kti == 0 and not with_bias_v),
                                 stop=(kti == KT_TILES - 1))
            for h in range(2):
                nc.vector.tensor_scalar_max(va[:, tb, h, 0:HD],
                                            vps[:, h * HD:(h + 1) * HD], 0.0)

        # gather one qc's sliver exchange into ctxt[b]
        def gather(b):
            # SWDGE so a collective-gated wait never blocks the HWDGE queues
            for i in range(NCORES):
                nc.gpsimd.dma_start(out=ctxt[b][:, i, :], in_=a2a_out[b][i])

        # output projection for one 128-token block of this core's share
        def outproj_block(b, tb, tag="proj"):
            for ec in range(D // 512):
                ps = psum.tile([P, 512], f32, tag=tag,
                               bufs=(3 if tag == "ctx" else 1),
                               name=f"po{b}_{tb}_{ec}")
                if with_bias_o:
                    nc.tensor.matmul(ps[:], ones[:], bo_s[:, ts(ec, 512)],
                                     start=True, stop=False)
                for kti in range(KT_TILES):
                    nc.tensor.matmul(ps[:], ctxt[b][:, kti, ts(tb, P)],
                                     wo_s[:, kti, ts(ec, 512)],
                                     start=(kti == 0 and not with_bias_o),
                                     stop=(kti == KT_TILES - 1))
                osb = osb_p.tile([P, 512], f32, tag="osb")
                nc.vector.tensor_scalar_max(osb[:], ps[:], 0.0)
                nc.sync.dma_start(out=out.ap()[ds(b * CH + tb * P, P), ts(ec, 512)],
                                  in_=osb[:])

        # attention for one batch; fillers[i] emitted at fractional positions
        def attention(b, fillers, positions):
            order = sorted(range(len(fillers)), key=lambda i: positions[i])
            fi = 0
            n_iter = SB_Q * KB
            it = 0
            for qc in range(SB_Q):
                qsl = ds(b * S + qc * 512, 512)
                cps = [psum.tile([P, 512], f32, tag="ctx", bufs=3,
                                 name=f"cps{b}_{qc}_{h}") for h in range(2)]
                for kb in range(KB):
                    ksl = ds(b * S + kb * P, P)
                    sps = psum.tile([P, 2, 512], f32, tag="sc", bufs=2)
                    for h in range(2):
                        nc.tensor.matmul(sps[:, h], kt[h][:, ksl], qt[h][:, qsl],
                                         start=True, stop=True)
                    pt = ptp.tile([P, 2, 512], DT, tag="p")
                    nc.scalar.activation(pt[:], sps[:], AF.Exp, scale=0.125)
                    for h in range(2):
                        nc.tensor.matmul(cps[h][:], va[:, b * KB + kb, h], pt[:, h],
                                         start=(kb == 0), stop=(kb == KB - 1))
                    it += 1
                    while fi < len(order) and positions[order[fi]] * n_iter < it:
                        fillers[order[fi]]()
                        fi += 1
                # normalize in SBUF after a fast PSUM-releasing copy
                for h in range(2):
                    cfull = nrm.tile([P, 512], f32, tag="cf")
                    nc.vector.tensor_copy(cfull[:], cps[h][:])
                    recb = nrm.tile([HD, 512], f32, tag="recb")
                    nc.vector.reciprocal(recb[:], cfull[HD:P, :])
                    csb = nrm.tile([HD, 512], DT, tag="csb")
                    nc.vector.tensor_tensor(csb[:], cfull[0:HD, :], recb[:],
                                            mybir.AluOpType.mult)
                    # scatter the two 256-token half-chunks (dest cores 2qc, 2qc+1)
                    nc.sync.dma_start(
                        out=a2a_in[b][2 * qc:2 * qc + 2, h * HD:(h + 1) * HD, :]
                            .rearrange("j p c -> p j c"),
                        in_=csb[:].rearrange("p (j c) -> p j c", j=2))
            nc.gpsimd.collective_compute(
                "AllToAll", mybir.AluOpType.bypass,
                replica_groups=[list(range(NCORES))],
                ins=[a2a_in[b].opt()], outs=[a2a_out[b].opt()],
            )
            for i in order[fi:]:
                fillers[i]()

        # ================= schedule =================
        for qcg in range(SB_Q):
            proj_qk(qcg, wq_s, bq_s, qt, with_bias_qk, tag="ctx")
            proj_qk(qcg, wk_s, bk_s, kt, with_bias_qk, tag="ctx")
            for tb in range(4 * qcg, 4 * qcg + 4):
                proj_v(tb, tag="ctx")

        # batch-0 attention with batch-1 projections as filler
        fillers, pos = [], []
        for qcg in range(SB_Q, 2 * SB_Q):
            fillers.append(lambda q=qcg: proj_qk(q, wq_s, bq_s, qt, with_bias_qk, "proj"))
            fillers.append(lambda q=qcg: proj_qk(q, wk_s, bk_s, kt, with_bias_qk, "proj"))
            for tb in range(4 * qcg, 4 * qcg + 4):
                fillers.append(lambda t=tb: proj_v(t, "proj"))
        pos = [(i + 0.5) / len(fillers) for i in range(len(fillers))]
        attention(0, fillers, pos)

        # batch-1 attention; fillers: gather batch-0 ctx + project batch-0
        fillers = [lambda: gather(0),
                   lambda: outproj_block(0, 0),
                   lambda: outproj_block(0, 1)]
        pos = [0.30, 0.40, 0.70]
        attention(1, fillers, pos)

        # tail: gather batch-1 ctx + project its two token blocks
        gather(1)
        outproj_block(1, 0, tag="ctx")
        outproj_block(1, 1, tag="ctx")

    nc.compile()
    return nc


def _get(with_bias_v, with_bias_o, with_bias_qk):
    key = (with_bias_v, with_bias_o, with_bias_qk)
    if key not in _CACHE:
        _CACHE[key] = _build(*key)
    return _CACHE[key]


def kernel(x, Wq, bq, Wk, bk, Wv, bv, Wo, bo):
    global LAST_RESULTS
    from concourse.bass_utils import run_bass_kernel_spmd

    x = np.asarray(x, dtype=np.float32)
    Wq, Wk, Wv, Wo = (np.asarray(w, dtype=np.float32) for w in (Wq, Wk, Wv, Wo))
    bq, bk, bv, bo = (np.asarray(v, dtype=np.float32) for v in (bq, bk, bv, bo))

    wb_qk = bool(np.any(bq) or np.any(bk))
    wb_v = bool(np.any(bv))
    wb_o = bool(np.any(bo))
    nc = _get(wb_v, wb_o, wb_qk)

    xT = np.ascontiguousarray(x.reshape(T, D).astype(_bf).T)
    Wq16 = Wq.astype(_bf)
    Wk16 = Wk.astype(_bf)
    Wv16 = Wv.astype(_bf)
    Wo16 = np.ascontiguousarray(Wo.astype(_bf))
    bv16 = bv.astype(_bf)
    bo16 = np.ascontiguousarray(bo.astype(_bf).reshape(1, D))

    in_maps = []
    for c in range(NCORES):
        cs = slice(c * DC, (c + 1) * DC)
        in_maps.append({
            "xT": xT,
            "wq": np.ascontiguousarray(Wq16[:, cs]),
            "wk": np.ascontiguousarray(Wk16[:, cs]),
            "wv": np.ascontiguousarray(Wv16[:, cs]),
            "wo": Wo16,
            "bqv": np.ascontiguousarray(bq[cs].reshape(DC, 1)),
            "bkv": np.ascontiguousarray(bk[cs].reshape(DC, 1)),
            "bvv": np.ascontiguousarray(bv16[cs].reshape(1, DC)),
            "bov": bo16,
        })

    kw = {}
    if PROFILE:
        kw = dict(trace=True, trace_cores=[0])
    res = run_bass_kernel_spmd(nc, in_maps, core_ids=list(range(NCORES)), **kw)
    LAST_RESULTS = res

    # core j's out rows: 0:CH -> batch-0 tokens [j*CH, (j+1)*CH), CH:2CH ->
    # batch-1 tokens [S + j*CH, ...)
    full = np.empty((T, D), np.float32)
    for j in range(NCORES):
        o = res.results[j]["out"]
        full[j * CH:(j + 1) * CH] = o[0:CH]
        full[S + j * CH:S + (j + 1) * CH] = o[CH:2 * CH]
    return np.ascontiguousarray(full.reshape(B, S, D))
